# revision 13
# baseline (speedup 1.0000x reference)
"""Multi-head attention forward on 8 Trainium2 NeuronCores (Bass/Tile).

Problem: B=4, L=2048, D=1024, H=16 heads, DV=64.
  out = softmax((x_q Wq^T + bq)(x_k Wk^T + bk)^T / sqrt(DV)) (x_v Wv^T + bv) Wc^T + bc

Sharding (8 cores): core c handles batch b = c//2 and head-group g = c%2
(8 heads = 512 of the 1024 projection columns). Each core produces a
full-shape [L, D] partial of the output projection (contraction over its
512 attention-output dims); the host sums the two partials per batch and
adds bc.

Per-core device pipeline (all matmuls fp32r = full-rate fp32):
  1. QT = (Wq_g x^T) [512, 2048], KT likewise; V natural [2048, 8*65] with a
     ones column per head (rowsum trick).
  2. Per (head, q-half): scores^T tile [k=128, q=1024] in PSUM ->
     ACT exp (scale=1/8) -> SBUF; AV matmul accumulates [65, 1024] in PSUM
     over 16 k-tiles; row 64 = softmax denominator. DVE normalizes into
     attnT [512, 2048].
  3. Output projection: out_partial[l, n] accumulated over 4 d-tiles.
"""

import numpy as np

import concourse.bacc as bacc
import concourse.mybir as mybir
from concourse.tile import TileContext
from concourse.bass_utils import run_bass_kernel_spmd

B, L, D, H = 4, 2048, 1024, 16
DV = 64
HPC = 8           # heads per core
OC = HPC * DV     # 512 projection cols per core
NCORES = 8

F32 = mybir.dt.float32
F32R = mybir.dt.float32r
EXP = mybir.ActivationFunctionType.Exp

_CACHE = {}


def _build():
    nc = bacc.Bacc("TRN2", target_bir_lowering=False, debug=False,
                   num_devices=NCORES)

    xtq = nc.dram_tensor("XTQ", [D, L], F32R, kind="ExternalInput")
    xtk = nc.dram_tensor("XTK", [D, L], F32R, kind="ExternalInput")
    xtv = nc.dram_tensor("XTV", [D, L], F32R, kind="ExternalInput")
    wqt = nc.dram_tensor("WQT", [D, OC], F32R, kind="ExternalInput")
    wkt = nc.dram_tensor("WKT", [D, OC], F32R, kind="ExternalInput")
    wvt = nc.dram_tensor("WVT", [D, OC], F32R, kind="ExternalInput")
    wct = nc.dram_tensor("WCT", [OC, D], F32R, kind="ExternalInput")
    bq = nc.dram_tensor("BQ", [OC], F32, kind="ExternalInput")
    bk = nc.dram_tensor("BK", [OC], F32, kind="ExternalInput")
    bv = nc.dram_tensor("BV", [OC], F32, kind="ExternalInput")
    out = nc.dram_tensor("OUT", [L, D], F32, kind="ExternalOutput")

    NI = D // 128    # 8 contraction tiles for projections
    NM = OC // 128   # 4 o-tiles (head pairs)
    NLC = L // 512   # 4 l-chunks
    NLT = L // 128   # 16 l-tiles / k-tiles
    QW = 1024        # q-half width in stage 2

    with TileContext(nc) as tc:
        with (
            tc.tile_pool(name="qkt", bufs=2 * NM) as qkt_pool,
            tc.tile_pool(name="vext", bufs=NLT) as vext_pool,
            tc.tile_pool(name="attnt", bufs=NM) as attnt_pool,
        ):
            qt = [qkt_pool.tile([128, L], F32R, tag="qkt", name=f"qt{i}")
                  for i in range(NM)]
            kt = [qkt_pool.tile([128, L], F32R, tag="qkt", name=f"kt{i}")
                  for i in range(NM)]
            vext = [vext_pool.tile([128, HPC, DV + 1], F32R, name=f"vext{i}",
                                   tag="vext")
                    for i in range(NLT)]
            attnt = [attnt_pool.tile([128, L], F32R, name=f"attnt{i}",
                                     tag="attnt")
                     for i in range(NM)]

            # ---- stage 1: projections ----
            with (
                tc.tile_pool(name="w", bufs=2 * NI) as w_pool,
                tc.tile_pool(name="xt", bufs=2 * NI) as xt_pool,
                tc.tile_pool(name="bias", bufs=2 * NM + 1) as bias_pool,
                tc.tile_pool(name="ps1", bufs=4, space="PSUM") as ps1,
            ):
                qbias = []
                kbias = []
                for m in range(NM):
                    t = bias_pool.tile([128, 1], F32, tag="b1")
                    nc.sync.dma_start(
                        out=t, in_=bq[m * 128:(m + 1) * 128].unsqueeze(1))
                    qbias.append(t)
                    t = bias_pool.tile([128, 1], F32, tag="b1")
                    nc.sync.dma_start(
                        out=t, in_=bk[m * 128:(m + 1) * 128].unsqueeze(1))
                    kbias.append(t)
                vbias = bias_pool.tile([128, OC], F32, tag="bv", bufs=1)
                nc.sync.dma_start(
                    out=vbias, in_=bv[:].unsqueeze(0).to_broadcast((128, OC)))
                onesf = bias_pool.tile([128, HPC], F32, tag="ones", bufs=1)
                nc.vector.memset(onesf, 1.0)

                for dst, wsrc, xsrc, biases in (
                    (qt, wqt, xtq, qbias),
                    (kt, wkt, xtk, kbias),
                ):
                    w_tiles = []
                    for i in range(NI):
                        t = w_pool.tile([128, OC], F32R, tag="w")
                        nc.sync.dma_start(
                            out=t, in_=wsrc[i * 128:(i + 1) * 128, :])
                        w_tiles.append(t)
                    for lc in range(NLC):
                        x_tiles = []
                        for i in range(NI):
                            t = xt_pool.tile([128, 512], F32R, tag="xt")
                            nc.sync.dma_start(
                                out=t,
                                in_=xsrc[i * 128:(i + 1) * 128,
                                         lc * 512:(lc + 1) * 512])
                            x_tiles.append(t)
                        for m in range(NM):
                            ps = ps1.tile([128, 512], F32)
                            for i in range(NI):
                                nc.tensor.matmul(
                                    ps,
                                    lhsT=w_tiles[i][:, m * 128:(m + 1) * 128],
                                    rhs=x_tiles[i],
                                    start=(i == 0), stop=(i == NI - 1))
                            nc.vector.tensor_add(
                                dst[m][:, lc * 512:(lc + 1) * 512],
                                ps,
                                biases[m].to_broadcast((128, 512)))

                # V projection: natural layout + ones column per head
                wv_tiles = []
                for i in range(NI):
                    t = w_pool.tile([128, OC], F32R, tag="w")
                    nc.sync.dma_start(out=t, in_=wvt[i * 128:(i + 1) * 128, :])
                    wv_tiles.append(t)
                for lc in range(NLC):
                    x_tiles = []
                    for i in range(NI):
                        t = xt_pool.tile([128, 512], F32R, tag="xt")
                        nc.sync.dma_start(
                            out=t,
                            in_=xtv[i * 128:(i + 1) * 128,
                                    lc * 512:(lc + 1) * 512])
                        x_tiles.append(t)
                    for ls in range(4):
                        lt = lc * 4 + ls
                        ps = ps1.tile([128, 512], F32)
                        for i in range(NI):
                            nc.tensor.matmul(
                                ps,
                                lhsT=x_tiles[i][:, ls * 128:(ls + 1) * 128],
                                rhs=wv_tiles[i],
                                start=(i == 0), stop=(i == NI - 1))
                        nc.vector.tensor_add(
                            vext[lt][:, :, 0:DV],
                            ps.rearrange("p (h d) -> p h d", h=HPC),
                            vbias.rearrange("p (h d) -> p h d", h=HPC))
                        nc.vector.tensor_copy(vext[lt][:, :, DV], onesf)

            # ---- stage 2: attention ----
            with (
                tc.tile_pool(name="st", bufs=2, space="PSUM") as st_pool,
                tc.tile_pool(name="ot", bufs=2, space="PSUM") as ot_pool,
                tc.tile_pool(name="ex", bufs=4) as ex_pool,
                tc.tile_pool(name="rc", bufs=2) as rc_pool,
                tc.tile_pool(name="rb", bufs=2) as rb_pool,
                tc.tile_pool(name="rcd", bufs=2, space="DRAM") as rcd_pool,
            ):
                for h in range(HPC):
                    m, off = h // 2, (h % 2) * DV
                    for qc in range(L // QW):
                        ot = ot_pool.tile([DV + 1, QW], F32)
                        for k in range(NLT):
                            st = st_pool.tile([128, QW], F32)
                            for j in range(QW // 512):
                                nc.tensor.matmul(
                                    st[:, j * 512:(j + 1) * 512],
                                    lhsT=kt[m][off:off + DV,
                                               k * 128:(k + 1) * 128],
                                    rhs=qt[m][off:off + DV,
                                              qc * QW + j * 512:
                                              qc * QW + (j + 1) * 512],
                                    start=True, stop=True)
                            ex = ex_pool.tile([128, QW], F32R)
                            nc.scalar.activation(
                                out=ex, in_=st, func=EXP, scale=0.125)
                            for j in range(QW // 512):
                                nc.tensor.matmul(
                                    ot[:, j * 512:(j + 1) * 512],
                                    lhsT=vext[k][:, h, :],
                                    rhs=ex[:, j * 512:(j + 1) * 512],
                                    start=(k == 0), stop=(k == NLT - 1))
                        rc = rc_pool.tile([1, QW], F32)
                        nc.vector.reciprocal(rc, ot[DV:DV + 1, :])
                        rcd = rcd_pool.tile([QW], F32)
                        nc.sync.dma_start(out=rcd[:].unsqueeze(0), in_=rc)
                        rb = rb_pool.tile([DV, QW], F32)
                        nc.gpsimd.dma_start(
                            out=rb,
                            in_=rcd[:].unsqueeze(0).to_broadcast((DV, QW)))
                        nc.vector.tensor_mul(
                            attnt[m][off:off + DV, qc * QW:(qc + 1) * QW],
                            ot[0:DV, :],
                            rb)

            # ---- stage 3: output projection (partial) ----
            with (
                tc.tile_pool(name="ps3", bufs=4, space="PSUM") as ps3,
                tc.tile_pool(name="ob", bufs=4) as ob_pool,
                tc.tile_pool(name="wc", bufs=NM) as wc_pool,
            ):
                wc_tiles = []
                for dt in range(NM):
                    t = wc_pool.tile([128, D], F32R)
                    nc.sync.dma_start(out=t, in_=wct[dt * 128:(dt + 1) * 128, :])
                    wc_tiles.append(t)
                for lt in range(NLT):
                    for nck in range(2):
                        ps = ps3.tile([128, 512], F32)
                        for dt in range(NM):
                            nc.tensor.matmul(
                                ps,
                                lhsT=attnt[dt][:, lt * 128:(lt + 1) * 128],
                                rhs=wc_tiles[dt][:, nck * 512:(nck + 1) * 512],
                                start=(dt == 0), stop=(dt == NM - 1))
                        ob = ob_pool.tile([128, 512], F32)
                        nc.vector.tensor_copy(ob, ps)
                        nc.sync.dma_start(
                            out=out[lt * 128:(lt + 1) * 128,
                                    nck * 512:(nck + 1) * 512],
                            in_=ob)

    nc.compile()
    return nc


def _get_nc():
    if "nc" not in _CACHE:
        _CACHE["nc"] = _build()
    return _CACHE["nc"]


def kernel(query, key, value, Wq, bq, Wk, bk, Wv, bv, Wc, bc, **_unused):
    query = np.asarray(query, np.float32)
    key = np.asarray(key, np.float32)
    value = np.asarray(value, np.float32)
    Wq = np.asarray(Wq, np.float32)
    Wk = np.asarray(Wk, np.float32)
    Wv = np.asarray(Wv, np.float32)
    Wc = np.asarray(Wc, np.float32)
    bq = np.asarray(bq, np.float32)
    bk = np.asarray(bk, np.float32)
    bv = np.asarray(bv, np.float32)
    bc = np.asarray(bc, np.float32)

    nc = _get_nc()

    xtq = [np.ascontiguousarray(query[b].T) for b in range(B)]
    xtk = [np.ascontiguousarray(key[b].T) for b in range(B)]
    xtv = [np.ascontiguousarray(value[b].T) for b in range(B)]
    wqt_g = [np.ascontiguousarray(Wq[g * OC:(g + 1) * OC, :].T) for g in range(2)]
    wkt_g = [np.ascontiguousarray(Wk[g * OC:(g + 1) * OC, :].T) for g in range(2)]
    wvt_g = [np.ascontiguousarray(Wv[g * OC:(g + 1) * OC, :].T) for g in range(2)]
    wct_g = [np.ascontiguousarray(Wc[:, g * OC:(g + 1) * OC].T) for g in range(2)]

    in_maps = []
    for c in range(NCORES):
        b, g = c // 2, c % 2
        in_maps.append({
            "XTQ": xtq[b], "XTK": xtk[b], "XTV": xtv[b],
            "WQT": wqt_g[g], "WKT": wkt_g[g], "WVT": wvt_g[g],
            "WCT": wct_g[g],
            "BQ": np.ascontiguousarray(bq[g * OC:(g + 1) * OC]),
            "BK": np.ascontiguousarray(bk[g * OC:(g + 1) * OC]),
            "BV": np.ascontiguousarray(bv[g * OC:(g + 1) * OC]),
        })

    res = run_bass_kernel_spmd(nc, in_maps, core_ids=list(range(NCORES)),
                               **_CACHE.get("run_kwargs", {}))
    _CACHE["last_results"] = res

    outp = np.empty((B, L, D), np.float32)
    for b in range(B):
        outp[b] = res.results[2 * b]["OUT"] + res.results[2 * b + 1]["OUT"]
    outp += bc
    return outp
